# revision 6
# baseline (speedup 1.0000x reference)
"""DGCN diffusion-graph-conv kernel for 8 Trainium2 NeuronCores.

Math (per the reference):
    support S = D^-1/2 (adj+I)^T D^-1/2  with D = diag(rowsum(adj+I))
    x_m = T_m(S) x0  (Chebyshev recurrence, K=3 -> m=0..3)
    out = sum_m x_m @ W_m + bias

Folded Chebyshev coefficients:
    V0 = W0 - W2, V1 = W1 - 3*W3, V2 = 2*W2, V3 = 4*W3
    U_m = x0 @ V_m;  out = U0 + S*(U1 + S*(U2 + S*U3))   (Horner)

Mixed precision (data-parallel over batch, 4 batches/core):
    - U0 (carries the full-magnitude signal): bf16 matmuls, full PE rate.
    - U1..U3 projections and the three S-contractions: fp8e4 matmuls in
      DoubleRow mode (two contraction planes per PE cell -> ~1.7x rate).
      Their errors are contracted by S (sigma_2(S) ~ 0.05) before they
      reach the output; measured rel_max ~8e-3 vs the 2e-2 gate.
    - fp8 operands are pre-scaled by powers of two (x0 x4, V123 x2,
      S x256, h-domain x8) chosen so every U-projection PSUM lands
      directly in the fp8 "h domain" (scale 8) and evictions are plain
      copies; Horner evictions fold the 1/256 S-scale into one
      scalar_tensor_tensor.  All values stay below the TRN fp8e4 max
      of +-240.
"""

import numpy as np
import ml_dtypes

import concourse.bacc as bacc
import concourse.tile as tile
import concourse.mybir as mybir
from concourse.bass_utils import run_bass_kernel_spmd

F32 = mybir.dt.float32
BF16 = mybir.dt.bfloat16
F8 = mybir.dt.float8e4
AX = mybir.AxisListType
ALU = mybir.AluOpType
DR = mybir.MatmulPerfMode.DoubleRow

N_CORES = 8
B, N, D = 32, 512, 768
BL = B // N_CORES          # local batches per core = 4
BN = BL * N                # local rows = 2048
DT = D // 128              # 6 feature tiles (bf16 path)
DTP = DT // 2              # 3 feature plane-pairs (fp8 path)
JT = N // 128              # 4 node tiles
JTP = JT // 2              # 2 node plane-pairs
CB = 3                     # 256-wide column blocks per 768 output cols
VCOLS = 3 * D              # 2304 concatenated V123 columns

SX = 4.0                   # x0 fp8 scale
SV = 2.0                   # V123 fp8 scale (SX*SV = 8 = h-domain scale)
SS = 256.0                 # S fp8 scale

# U123 column chunks over the host-permuted 2304-wide Vcat:
#   [V1 e0:512 | V2 e0:512 | V3 e0:512 | V1 e512:768 , V2 e512:768 |
#    V3 e512:768]
# (m, cb0, ncb) per chunk; chunk width = 256*ncb
U123_CHUNKS = [
    (0, [(1, 0, 2)]),          # cols    0:512  -> u1 cb0-1
    (512, [(2, 0, 2)]),        # cols  512:1024 -> u2 cb0-1
    (1024, [(3, 0, 2)]),       # cols 1024:1536 -> u3 cb0-1
    (1536, [(1, 2, 1), (2, 2, 1)]),   # u1 cb2 + u2 cb2
    (2048, [(3, 2, 1)]),       # u3 cb2
]


def _build_program():
    nc = bacc.Bacc("TRN2", target_bir_lowering=False, debug=False,
                   num_devices=N_CORES)
    inp0_d = nc.dram_tensor("inp0", [D, BN], BF16, kind="ExternalInput").ap()
    inp8_d = nc.dram_tensor("inp8", [DTP * 128, 2 * BN], F8,
                            kind="ExternalInput").ap()
    v0_d = nc.dram_tensor("v0", [D, D], BF16, kind="ExternalInput").ap()
    v8_d = nc.dram_tensor("v8", [DTP * 128, 2 * VCOLS], F8,
                          kind="ExternalInput").ap()
    adj_d = nc.dram_tensor("adj", [N, N], F32, kind="ExternalInput").ap()
    bias_d = nc.dram_tensor("bias", [D], F32, kind="ExternalInput").ap()
    eye_d = nc.dram_tensor("eye", [128, 128], F32, kind="ExternalInput").ap()
    out_d = nc.dram_tensor("out", [BN, D], F32, kind="ExternalOutput").ap()
    dscr = nc.dram_tensor("dscr", [N], F32)

    with tile.TileContext(nc) as tc:
        with (
            tc.tile_pool(name="const", bufs=1) as constp,
            tc.tile_pool(name="sup", bufs=1) as supp,
            tc.tile_pool(name="x0", bufs=1) as x0p,
            tc.tile_pool(name="x8", bufs=1) as x8p,
            tc.tile_pool(name="v0", bufs=1) as v0p,
            tc.tile_pool(name="v8", bufs=1) as v8p,
            tc.tile_pool(name="u0", bufs=1) as u0p,
            tc.tile_pool(name="u12", bufs=1) as u12p,
            tc.tile_pool(name="u3", bufs=1) as u3p,
            tc.tile_pool(name="h", bufs=4) as hp,
            tc.tile_pool(name="stg", bufs=4) as stgp,
            tc.tile_pool(name="ps0", bufs=2, space="PSUM") as ps0p,
            tc.tile_pool(name="ps8", bufs=3, space="PSUM") as ps8p,
            tc.tile_pool(name="psh", bufs=3, space="PSUM") as pshp,
        ):
            # ---- input DMAs, first-needed first; fp8 operands (small)
            #      on the sync ring, bf16/f32 on the gpsimd ring ----
            v0t = [v0p.tile([128, D], BF16, name=f"v0_{dt}")
                   for dt in range(DT)]
            x0t = [x0p.tile([128, BN], BF16, name=f"x0_{dt}")
                   for dt in range(DT)]
            v8t = [v8p.tile([128, 2, VCOLS], F8, name=f"v8_{dtp}")
                   for dtp in range(DTP)]
            x8t = [x8p.tile([128, 2, BN], F8, name=f"x8_{dtp}")
                   for dtp in range(DTP)]
            # sync ring: v8 first halves -> x8 batch0 -> v8 rest -> x8 rest
            for dtp in range(DTP):
                r = slice(dtp * 128, (dtp + 1) * 128)
                for i in range(2):
                    nc.sync.dma_start(
                        v8t[dtp][:, i, 0:1152],
                        v8_d[r, i * VCOLS:i * VCOLS + 1152])
            for dtp in range(DTP):
                r = slice(dtp * 128, (dtp + 1) * 128)
                for i in range(2):
                    nc.sync.dma_start(
                        x8t[dtp][:, i, 0:512],
                        inp8_d[r, i * BN:i * BN + 512])
            for dtp in range(DTP):
                r = slice(dtp * 128, (dtp + 1) * 128)
                for i in range(2):
                    nc.sync.dma_start(
                        v8t[dtp][:, i, 1152:VCOLS],
                        v8_d[r, i * VCOLS + 1152:(i + 1) * VCOLS])
            for ck in range(1, 4):
                for dtp in range(DTP):
                    r = slice(dtp * 128, (dtp + 1) * 128)
                    for i in range(2):
                        nc.sync.dma_start(
                            x8t[dtp][:, i, ck * 512:(ck + 1) * 512],
                            inp8_d[r, i * BN + ck * 512:i * BN + (ck + 1) * 512])
            # gpsimd ring: eye, v0, x0 batch0, adj (support build's
            # dscr/dbc DMAs queue next), then x0 rest
            eye128 = constp.tile([128, 128], F32)
            nc.gpsimd.dma_start(eye128[:], eye_d[:])
            for dt in range(DT):
                nc.gpsimd.dma_start(v0t[dt][:],
                                    v0_d[dt * 128:(dt + 1) * 128, :])
                nc.gpsimd.dma_start(x0t[dt][:, 0:512],
                                    inp0_d[dt * 128:(dt + 1) * 128, 0:512])
            adjts = []
            for t in range(JT):
                adjt = supp.tile([128, N], F32, name=f"adjt{t}")
                nc.gpsimd.dma_start(adjt[:], adj_d[t * 128:(t + 1) * 128, :])
                adjts.append(adjt)
            bias_bc = constp.tile([128, D], F32)
            nc.gpsimd.dma_start(
                bias_bc[:], bias_d.unsqueeze(0).broadcast_to([128, D]))

            # ---- support matrix S^T (f32 build as baseline, then x256
            #      quantize into fp8 plane-pair tiles) ----
            dcols, dsqs = [], []
            for t in range(JT):
                rs = supp.tile([128, 1], F32, name=f"rs{t}", tag="rs", bufs=2)
                nc.vector.tensor_reduce(rs[:], adjts[t][:], axis=AX.X,
                                        op=ALU.add)
                nc.vector.tensor_scalar_add(rs[:], rs[:], 1.0)
                sq = supp.tile([128, 1], F32, name=f"sq{t}", tag="sq", bufs=2)
                nc.scalar.sqrt(sq[:], rs[:])
                dcol = supp.tile([128, 1], F32, name=f"dcol{t}")
                nc.vector.reciprocal(dcol[:], sq[:])
                dsq = supp.tile([128, 1], F32, name=f"dsq{t}")
                nc.vector.tensor_mul(dsq[:], dcol[:], dcol[:])
                nc.gpsimd.dma_start(dscr.ap()[t * 128:(t + 1) * 128], dcol[:])
                dcols.append(dcol)
                dsqs.append(dsq)
            dbc = constp.tile([128, N], F32)
            nc.gpsimd.dma_start(
                dbc[:], dscr.ap().unsqueeze(0).broadcast_to([128, N]))
            st8 = [supp.tile([128, 2, N], F8, name=f"st8_{jtp}")
                   for jtp in range(JTP)]
            for t in range(JT):
                s = supp.tile([128, N], F32, name=f"st{t}", tag="stf",
                              bufs=2)
                nc.vector.scalar_tensor_tensor(
                    s[:], adjts[t][:], dcols[t][:], dbc[:],
                    ALU.mult, ALU.mult)
                diagfix = supp.tile([128, 128], F32, name=f"dfix{t}",
                                    tag="dfix", bufs=2)
                nc.vector.tensor_scalar_mul(diagfix[:], eye128[:], dsqs[t][:])
                nc.vector.tensor_add(
                    s[:, t * 128:(t + 1) * 128],
                    s[:, t * 128:(t + 1) * 128], diagfix[:])
                nc.vector.tensor_scalar_mul(st8[t // 2][:, t % 2, :], s[:],
                                            SS)
            # late x0 chunks on the gpsimd ring, behind dscr/dbc
            for ck in range(1, 4):
                for dt in range(DT):
                    nc.gpsimd.dma_start(
                        x0t[dt][:, ck * 512:(ck + 1) * 512],
                        inp0_d[dt * 128:(dt + 1) * 128,
                               ck * 512:(ck + 1) * 512])
            # bf16 identity + 2048*bias[512:768] for the tail's
            # PE-side addend path (batch pair 1, cb2 groups)
            eye_bf = constp.tile([128, 128], BF16)
            nc.vector.tensor_copy(eye_bf[:], eye128[:])
            b2k = constp.tile([128, 256], F32)
            nc.vector.tensor_scalar_mul(b2k[:], bias_bc[:, 512:768], 2048.0)

            # ---- per-batch-pair U tiles ----
            # u0: [row128, batch-parity, 768] bf16 (holds U0 + bias)
            # u12: [row128, cb, batch-parity, 256] bf16 (holds 8*U_m)
            # u3/h: [row128, j-plane, cb, batch-parity, 256] fp8 (8*U3 / 8*h)
            u0t = [[u0p.tile([128, 2, D], BF16, name=f"u0_{bp}_{nt}")
                    for nt in range(JT)] for bp in range(2)]
            u12t = [[[u12p.tile([128, CB, 2, 256], BF16,
                                name=f"u{m}_{bp}_{nt}")
                      for nt in range(JT)] for m in (1, 2)]
                    for bp in range(2)]
            u3t = [[u3p.tile([128, 2, CB, 2, 256], F8,
                             name=f"u3_{bp}_{jtp}")
                    for jtp in range(JTP)] for bp in range(2)]

            def proj(b):
                """U123 (fp8 DoubleRow) + U0 (bf16) for batch b.

                For batch pair 1, the cb2 slices of u1/u2/u0 are stored in
                the 2048 (PSUM) domain so the Horner tail can add them via
                identity-matmul accumulation on the PE."""
                bp, h = b // 2, b % 2
                for nt in range(JT):
                    bt = b * JT + nt
                    rsl = slice(bt * 128, (bt + 1) * 128)
                    # U123: fp8 DoubleRow over host-permuted Vcat chunks
                    for c0, dests in U123_CHUNKS:
                        cw = 256 * sum(ncb for _, _, ncb in dests)
                        ps = ps8p.tile([128, 512], F32,
                                       name=f"p8_{bt}_{c0}", tag="ps8")
                        for dtp in range(DTP):
                            nc.tensor.matmul(
                                ps[:, 0:cw], x8t[dtp][:, :, rsl],
                                v8t[dtp][:, :, c0:c0 + cw],
                                start=(dtp == 0), stop=(dtp == DTP - 1),
                                perf_mode=DR)
                        off = 0
                        for m, cb0, ncb in dests:
                            w = 256 * ncb
                            src = ps[:, off:off + w]
                            if m == 3:
                                nc.scalar.copy(
                                    u3t[bp][nt // 2][
                                        :, nt % 2, cb0:cb0 + ncb, h, :],
                                    src)
                            elif bp == 1 and cb0 == 2:
                                nc.scalar.mul(
                                    u12t[bp][m - 1][nt][
                                        :, cb0:cb0 + ncb, h, :],
                                    src, SS)
                            else:
                                nc.scalar.copy(
                                    u12t[bp][m - 1][nt][
                                        :, cb0:cb0 + ncb, h, :],
                                    src)
                            off += w
                    # U0: full-rate bf16, cols 768 in (512, 256) chunks
                    for c0, cw in ((0, 512), (512, 256)):
                        ps = ps0p.tile([128, 512], F32,
                                       name=f"p0_{bt}_{c0}", tag="ps0")
                        for dt in range(DT):
                            nc.tensor.matmul(
                                ps[:, 0:cw], x0t[dt][:, rsl],
                                v0t[dt][:, c0:c0 + cw],
                                start=(dt == 0), stop=(dt == DT - 1))
                        if bp == 1 and c0 == 512:
                            nc.vector.scalar_tensor_tensor(
                                u0t[bp][nt][:, h, c0:c0 + cw], ps[:, 0:cw],
                                2048.0, b2k[:], ALU.mult, ALU.add)
                        else:
                            nc.vector.tensor_add(
                                u0t[bp][nt][:, h, c0:c0 + cw], ps[:, 0:cw],
                                bias_bc[:, c0:c0 + cw])

            def horner(bp):
                """out = U0 + S*(U1 + S*(U2 + S*U3)) for batch pair bp."""
                hsrc = u3t[bp]
                for step, madd in ((2, 2), (1, 1), (0, 0)):
                    hdst = None
                    if step > 0:
                        hdst = [hp.tile([128, 2, CB, 2, 256], F8,
                                        name=f"h_{bp}_{step}_{jtp}",
                                        tag="h")
                                for jtp in range(JTP)]
                    for nt in range(JT):
                        nsl = slice(nt * 128, (nt + 1) * 128)
                        for cb in range(CB):
                            # batch pair 1 cb2: U-addend joins via
                            # identity-matmul so the evict is a pure scaled
                            # copy that runs on the Scalar engine (keeps
                            # the tail off the DVE critical path)
                            pe_add = bp == 1 and cb == 2
                            ph = pshp.tile([128, 2, 256], F32,
                                           name=f"ph_{bp}_{step}_{nt}_{cb}",
                                           tag="psh")
                            for jtp in range(JTP):
                                nc.tensor.matmul(
                                    ph[:], st8[jtp][:, :, nsl],
                                    hsrc[jtp][:, :, cb, :, :],
                                    start=(jtp == 0),
                                    stop=(jtp == JTP - 1 and not pe_add),
                                    perf_mode=DR)
                            if pe_add:
                                addend = (
                                    u12t[bp][madd - 1][nt][:, 2, :, :]
                                    if step > 0 else
                                    u0t[bp][nt][:, :, 512:768])
                                nc.tensor.matmul(
                                    ph[:], eye_bf[:], addend,
                                    start=False, stop=True)
                            if step > 0:
                                hd = hdst[nt // 2][:, nt % 2, cb, :, :]
                                if pe_add:
                                    # h_new = psum/256  (fp8 out, ACT)
                                    nc.scalar.mul(hd, ph[:], 1.0 / SS)
                                else:
                                    # h_new = psum/256 + 8*U_madd (DVE)
                                    nc.vector.scalar_tensor_tensor(
                                        hd, ph[:], 1.0 / SS,
                                        u12t[bp][madd - 1][nt][:, cb, :, :],
                                        ALU.mult, ALU.add)
                            else:
                                so = stgp.tile([128, 2, 256], F32,
                                               name=f"so_{bp}_{nt}_{cb}",
                                               tag="outst")
                                if pe_add:
                                    nc.scalar.mul(so[:], ph[:],
                                                  1.0 / (SS * SX * SV))
                                else:
                                    # out = psum/2048 + (U0 + bias)
                                    nc.vector.scalar_tensor_tensor(
                                        so[:], ph[:], 1.0 / (SS * SX * SV),
                                        u0t[bp][nt][:, :,
                                                    cb * 256:(cb + 1) * 256],
                                        ALU.mult, ALU.add)
                                r0 = (2 * bp * JT + nt) * 128
                                nc.sync.dma_start(
                                    out_d.rearrange(
                                        "(x p) e -> p x e", p=128)[
                                        :, r0 // 128:r0 // 128 + 5:4,
                                        cb * 256:(cb + 1) * 256],
                                    so[:])
                    hsrc = hdst

            proj(0)
            proj(1)
            proj(2)
            horner(0)
            proj(3)
            horner(1)
    nc.compile()
    return nc


_CACHE = {}


def _get_program():
    if "nc" not in _CACHE:
        _CACHE["nc"] = _build_program()
    return _CACHE["nc"]


def _e4(x):
    return np.clip(x, -240.0, 240.0).astype(ml_dtypes.float8_e4m3)


def _planepair(a):
    """[768, X] -> [384, 2X]: row = dtp*128+p, col = plane*X + x."""
    x = a.shape[1]
    return np.ascontiguousarray(
        a.reshape(DTP, 2, 128, x).transpose(0, 2, 1, 3).reshape(DTP * 128,
                                                                2 * x))


def make_in_maps(inputs, adj, weights, biases):
    inputs = np.ascontiguousarray(inputs, dtype=np.float32)
    adj = np.ascontiguousarray(adj, dtype=np.float32)
    weights = np.ascontiguousarray(weights, dtype=np.float32)
    biases = np.ascontiguousarray(biases, dtype=np.float32)
    assert inputs.shape == (B, N, D)
    assert adj.shape == (N, N)
    assert weights.shape == (D * 4, D)
    assert biases.shape == (D,)
    eye = np.eye(128, dtype=np.float32)

    Wm = weights.reshape(D, 4, D).transpose(1, 0, 2)  # [m, d, e]
    V0 = Wm[0] - Wm[2]
    V1 = Wm[1] - 3.0 * Wm[3]
    V2 = 2.0 * Wm[2]
    V3 = 4.0 * Wm[3]
    v0 = np.ascontiguousarray(V0).astype(ml_dtypes.bfloat16)
    vcat = np.concatenate(
        [V1[:, 0:512], V2[:, 0:512], V3[:, 0:512],
         V1[:, 512:768], V2[:, 512:768], V3[:, 512:768]], axis=1) * SV
    v8 = _planepair(_e4(vcat))

    in_maps = []
    for c in range(N_CORES):
        x0T = np.ascontiguousarray(
            inputs[c * BL:(c + 1) * BL].reshape(BN, D).T)
        in_maps.append({
            "inp0": x0T.astype(ml_dtypes.bfloat16),
            "inp8": _planepair(_e4(x0T * SX)),
            "v0": v0,
            "v8": v8,
            "adj": adj,
            "bias": biases,
            "eye": eye,
        })
    return in_maps


def kernel(inputs, adj, weights, biases):
    nc = _get_program()
    in_maps = make_in_maps(inputs, adj, weights, biases)
    res = run_bass_kernel_spmd(nc, in_maps, list(range(N_CORES)))
    out = np.concatenate(
        [res.results[c]["out"].reshape(BL, N, D) for c in range(N_CORES)],
        axis=0)
    return out


# revision 8
# speedup vs baseline: 1.1139x; 1.1139x over previous
"""DGCN diffusion-graph-conv kernel for 8 Trainium2 NeuronCores.

Math (per the reference):
    support S = D^-1/2 (adj+I)^T D^-1/2  with D = diag(rowsum(adj+I))
    x_m = T_m(S) x0  (Chebyshev recurrence, K=3 -> m=0..3)
    out = sum_m x_m @ W_m + bias

Folded Chebyshev coefficients:
    V0 = W0 - W2, V1 = W1 - 3*W3, V2 = 2*W2, V3 = 4*W3
    U_m = x0 @ V_m;  out = U0 + S*(U1 + S*(U2 + S*U3))   (Horner)

Mixed precision (data-parallel over batch, 4 batches/core):
    - U0 (carries the full-magnitude signal): bf16 matmuls, full PE rate.
    - U1..U3 projections and the three S-contractions: fp8e4 matmuls in
      DoubleRow mode (two contraction planes per PE cell -> ~1.7x rate).
      Their errors are contracted by S (sigma_2(S) ~ 0.05) before they
      reach the output; measured rel_max ~8e-3 vs the 2e-2 gate.
    - fp8 operands are pre-scaled by powers of two (x0 x4, V123 x2,
      S x256, h-domain x8) chosen so every U-projection PSUM lands
      directly in the fp8 "h domain" (scale 8) and evictions are plain
      copies; Horner evictions fold the 1/256 S-scale into one
      scalar_tensor_tensor.  All values stay below the TRN fp8e4 max
      of +-240.
"""

import numpy as np
import ml_dtypes

import concourse.bacc as bacc
import concourse.tile as tile
import concourse.mybir as mybir
from concourse.bass_utils import run_bass_kernel_spmd

F32 = mybir.dt.float32
BF16 = mybir.dt.bfloat16
F8 = mybir.dt.float8e4
AX = mybir.AxisListType
ALU = mybir.AluOpType
DR = mybir.MatmulPerfMode.DoubleRow

N_CORES = 8
B, N, D = 32, 512, 768
BL = B // N_CORES          # local batches per core = 4
BN = BL * N                # local rows = 2048
DT = D // 128              # 6 feature tiles (bf16 path)
DTP = DT // 2              # 3 feature plane-pairs (fp8 path)
JT = N // 128              # 4 node tiles
JTP = JT // 2              # 2 node plane-pairs
CB = 3                     # 256-wide column blocks per 768 output cols
VCOLS = 3 * D              # 2304 concatenated V123 columns

SX = 4.0                   # x0 fp8 scale
SV = 2.0                   # V123 fp8 scale (SX*SV = 8 = h-domain scale)
SS = 256.0                 # S fp8 scale

# U123 column chunks over the host-permuted 2304-wide Vcat:
#   [V1 e0:512 | V2 e0:512 | V3 e0:512 | V1 e512:768 , V2 e512:768 |
#    V3 e512:768]
# (m, cb0, ncb) per chunk; chunk width = 256*ncb
U123_CHUNKS = [
    (0, [(1, 0, 2)]),          # cols    0:512  -> u1 cb0-1
    (512, [(2, 0, 2)]),        # cols  512:1024 -> u2 cb0-1
    (1024, [(3, 0, 2)]),       # cols 1024:1536 -> u3 cb0-1
    (1536, [(1, 2, 1), (2, 2, 1)]),   # u1 cb2 + u2 cb2
    (2048, [(3, 2, 1)]),       # u3 cb2
]


def _build_program():
    nc = bacc.Bacc("TRN2", target_bir_lowering=False, debug=False,
                   num_devices=N_CORES)
    inp0_d = nc.dram_tensor("inp0", [D, BN], BF16, kind="ExternalInput").ap()
    inp8_d = nc.dram_tensor("inp8", [DTP * 128, 2 * BN], F8,
                            kind="ExternalInput").ap()
    v0_d = nc.dram_tensor("v0", [D, D], BF16, kind="ExternalInput").ap()
    v8_d = nc.dram_tensor("v8", [DTP * 128, 2 * VCOLS], F8,
                          kind="ExternalInput").ap()
    adj_d = nc.dram_tensor("adj", [N, N], F32, kind="ExternalInput").ap()
    bias_d = nc.dram_tensor("bias", [D], F32, kind="ExternalInput").ap()
    eye_d = nc.dram_tensor("eye", [128, 128], F32, kind="ExternalInput").ap()
    out_d = nc.dram_tensor("out", [BN, D], F32, kind="ExternalOutput").ap()
    dscr = nc.dram_tensor("dscr", [N], F32)

    with tile.TileContext(nc) as tc:
        with (
            tc.tile_pool(name="const", bufs=1) as constp,
            tc.tile_pool(name="sup", bufs=1) as supp,
            tc.tile_pool(name="x0", bufs=1) as x0p,
            tc.tile_pool(name="x8", bufs=1) as x8p,
            tc.tile_pool(name="v0", bufs=1) as v0p,
            tc.tile_pool(name="v8", bufs=1) as v8p,
            tc.tile_pool(name="u0", bufs=1) as u0p,
            tc.tile_pool(name="u12", bufs=1) as u12p,
            tc.tile_pool(name="u3", bufs=1) as u3p,
            tc.tile_pool(name="h", bufs=4) as hp,
            tc.tile_pool(name="stg", bufs=4) as stgp,
            tc.tile_pool(name="ps0", bufs=2, space="PSUM") as ps0p,
            tc.tile_pool(name="ps8", bufs=3, space="PSUM") as ps8p,
            tc.tile_pool(name="psh", bufs=3, space="PSUM") as pshp,
        ):
            # ---- input DMAs, first-needed first; fp8 operands (small)
            #      on the sync ring, bf16/f32 on the gpsimd ring ----
            v0t = [v0p.tile([128, D], BF16, name=f"v0_{dt}")
                   for dt in range(DT)]
            x0t = [x0p.tile([128, BN], BF16, name=f"x0_{dt}")
                   for dt in range(DT)]
            v8t = [v8p.tile([128, 2, VCOLS], F8, name=f"v8_{dtp}")
                   for dtp in range(DTP)]
            x8t = [x8p.tile([128, 2, BN], F8, name=f"x8_{dtp}")
                   for dtp in range(DTP)]
            eye128 = constp.tile([128, 128], F32)
            nc.gpsimd.dma_start(eye128[:], eye_d[:])
            adjts = []
            for t in range(JT):
                adjt = supp.tile([128, N], F32, name=f"adjt{t}")
                nc.gpsimd.dma_start(adjt[:], adj_d[t * 128:(t + 1) * 128, :])
                adjts.append(adjt)
            # sync ring carries the bulk inputs, first-needed first:
            # v8 first-halves, x8 batch0, v0, x0 batch0, v8 rest, then
            # the later batches
            for dtp in range(DTP):
                r = slice(dtp * 128, (dtp + 1) * 128)
                for i in range(2):
                    nc.sync.dma_start(
                        v8t[dtp][:, i, 0:1152],
                        v8_d[r, i * VCOLS:i * VCOLS + 1152])
            for dtp in range(DTP):
                r = slice(dtp * 128, (dtp + 1) * 128)
                for i in range(2):
                    nc.sync.dma_start(
                        x8t[dtp][:, i, 0:512],
                        inp8_d[r, i * BN:i * BN + 512])
            for dt in range(DT):
                nc.sync.dma_start(v0t[dt][:],
                                  v0_d[dt * 128:(dt + 1) * 128, :])
                nc.sync.dma_start(x0t[dt][:, 0:512],
                                  inp0_d[dt * 128:(dt + 1) * 128, 0:512])
            for dtp in range(DTP):
                r = slice(dtp * 128, (dtp + 1) * 128)
                for i in range(2):
                    nc.sync.dma_start(
                        v8t[dtp][:, i, 1152:VCOLS],
                        v8_d[r, i * VCOLS + 1152:(i + 1) * VCOLS])
            for ck in range(1, 4):
                for dtp in range(DTP):
                    r = slice(dtp * 128, (dtp + 1) * 128)
                    for i in range(2):
                        eng = nc.gpsimd if ck == 3 else nc.sync
                        eng.dma_start(
                            x8t[dtp][:, i, ck * 512:(ck + 1) * 512],
                            inp8_d[r, i * BN + ck * 512:i * BN + (ck + 1) * 512])
                for dt in range(DT):
                    nc.sync.dma_start(
                        x0t[dt][:, ck * 512:(ck + 1) * 512],
                        inp0_d[dt * 128:(dt + 1) * 128,
                               ck * 512:(ck + 1) * 512])
            bias_bc = constp.tile([128, D], F32)
            nc.gpsimd.dma_start(
                bias_bc[:], bias_d.unsqueeze(0).broadcast_to([128, D]))

            # ---- support matrix S^T (f32 build as baseline, then x256
            #      quantize into fp8 plane-pair tiles) ----
            dcols, dsqs = [], []
            for t in range(JT):
                rs = supp.tile([128, 1], F32, name=f"rs{t}", tag="rs", bufs=2)
                nc.vector.tensor_reduce(rs[:], adjts[t][:], axis=AX.X,
                                        op=ALU.add)
                nc.vector.tensor_scalar_add(rs[:], rs[:], 1.0)
                sq = supp.tile([128, 1], F32, name=f"sq{t}", tag="sq", bufs=2)
                nc.scalar.sqrt(sq[:], rs[:])
                dcol = supp.tile([128, 1], F32, name=f"dcol{t}")
                nc.vector.reciprocal(dcol[:], sq[:])
                dsq = supp.tile([128, 1], F32, name=f"dsq{t}")
                nc.vector.tensor_mul(dsq[:], dcol[:], dcol[:])
                nc.gpsimd.dma_start(dscr.ap()[t * 128:(t + 1) * 128], dcol[:])
                dcols.append(dcol)
                dsqs.append(dsq)
            dbc = constp.tile([128, N], F32)
            nc.gpsimd.dma_start(
                dbc[:], dscr.ap().unsqueeze(0).broadcast_to([128, N]))
            st8 = [supp.tile([128, 2, N], F8, name=f"st8_{jtp}")
                   for jtp in range(JTP)]
            for t in range(JT):
                s = supp.tile([128, N], F32, name=f"st{t}", tag="stf",
                              bufs=2)
                nc.vector.scalar_tensor_tensor(
                    s[:], adjts[t][:], dcols[t][:], dbc[:],
                    ALU.mult, ALU.mult)
                diagfix = supp.tile([128, 128], F32, name=f"dfix{t}",
                                    tag="dfix", bufs=2)
                nc.vector.tensor_scalar_mul(diagfix[:], eye128[:], dsqs[t][:])
                nc.vector.tensor_add(
                    s[:, t * 128:(t + 1) * 128],
                    s[:, t * 128:(t + 1) * 128], diagfix[:])
                nc.vector.tensor_scalar_mul(st8[t // 2][:, t % 2, :], s[:],
                                            SS)
            # bf16 identity + 2048*bias[512:768] for the tail's
            # PE-side addend path (batch pair 1, cb2 groups)
            eye_bf = constp.tile([128, 128], BF16)
            nc.vector.tensor_copy(eye_bf[:], eye128[:])
            b2k = constp.tile([128, 256], F32)
            nc.vector.tensor_scalar_mul(b2k[:], bias_bc[:, 512:768], 2048.0)

            # ---- per-batch-pair U tiles ----
            # u0: [row128, batch-parity, 768] bf16 (holds U0 + bias)
            # u12: [row128, cb, batch-parity, 256] bf16 (holds 8*U_m)
            # u3/h: [row128, j-plane, cb, batch-parity, 256] fp8 (8*U3 / 8*h)
            u0t = [[u0p.tile([128, 2, D], BF16, name=f"u0_{bp}_{nt}")
                    for nt in range(JT)] for bp in range(2)]
            u12t = [[[u12p.tile([128, CB, 2, 256], BF16,
                                name=f"u{m}_{bp}_{nt}")
                      for nt in range(JT)] for m in (1, 2)]
                    for bp in range(2)]
            u3t = [[u3p.tile([128, 2, CB, 2, 256], F8,
                             name=f"u3_{bp}_{jtp}")
                    for jtp in range(JTP)] for bp in range(2)]

            def proj(b):
                """U123 (fp8 DoubleRow) + U0 (bf16) for batch b.

                For batch pair 1, the cb2 slices of u1/u2/u0 are stored in
                the 2048 (PSUM) domain so the Horner tail can add them via
                identity-matmul accumulation on the PE."""
                bp, h = b // 2, b % 2
                for nt in range(JT):
                    bt = b * JT + nt
                    rsl = slice(bt * 128, (bt + 1) * 128)
                    # U123: fp8 DoubleRow over host-permuted Vcat chunks
                    for c0, dests in U123_CHUNKS:
                        cw = 256 * sum(ncb for _, _, ncb in dests)
                        ps = ps8p.tile([128, 512], F32,
                                       name=f"p8_{bt}_{c0}", tag="ps8")
                        for dtp in range(DTP):
                            nc.tensor.matmul(
                                ps[:, 0:cw], x8t[dtp][:, :, rsl],
                                v8t[dtp][:, :, c0:c0 + cw],
                                start=(dtp == 0), stop=(dtp == DTP - 1),
                                perf_mode=DR)
                        off = 0
                        for m, cb0, ncb in dests:
                            w = 256 * ncb
                            src = ps[:, off:off + w]
                            if m == 3:
                                nc.scalar.copy(
                                    u3t[bp][nt // 2][
                                        :, nt % 2, cb0:cb0 + ncb, h, :],
                                    src)
                            elif bp == 1 and cb0 == 2:
                                nc.scalar.mul(
                                    u12t[bp][m - 1][nt][
                                        :, cb0:cb0 + ncb, h, :],
                                    src, SS)
                            else:
                                nc.scalar.copy(
                                    u12t[bp][m - 1][nt][
                                        :, cb0:cb0 + ncb, h, :],
                                    src)
                            off += w
                    # U0: full-rate bf16, cols 768 in (512, 256) chunks
                    for c0, cw in ((0, 512), (512, 256)):
                        ps = ps0p.tile([128, 512], F32,
                                       name=f"p0_{bt}_{c0}", tag="ps0")
                        for dt in range(DT):
                            nc.tensor.matmul(
                                ps[:, 0:cw], x0t[dt][:, rsl],
                                v0t[dt][:, c0:c0 + cw],
                                start=(dt == 0), stop=(dt == DT - 1))
                        if bp == 1 and c0 == 512:
                            nc.vector.scalar_tensor_tensor(
                                u0t[bp][nt][:, h, c0:c0 + cw], ps[:, 0:cw],
                                2048.0, b2k[:], ALU.mult, ALU.add)
                        else:
                            nc.vector.tensor_add(
                                u0t[bp][nt][:, h, c0:c0 + cw], ps[:, 0:cw],
                                bias_bc[:, c0:c0 + cw])

            def horner(bp):
                """out = U0 + S*(U1 + S*(U2 + S*U3)) for batch pair bp."""
                hsrc = u3t[bp]
                for step, madd in ((2, 2), (1, 1), (0, 0)):
                    hdst = None
                    if step > 0:
                        hdst = [hp.tile([128, 2, CB, 2, 256], F8,
                                        name=f"h_{bp}_{step}_{jtp}",
                                        tag="h")
                                for jtp in range(JTP)]
                    for nt in range(JT):
                        nsl = slice(nt * 128, (nt + 1) * 128)
                        for cb in range(CB):
                            # batch pair 1 cb2: U-addend joins via
                            # identity-matmul so the evict is a pure scaled
                            # copy that runs on the Scalar engine (keeps
                            # the tail off the DVE critical path)
                            pe_add = bp == 1 and cb == 2
                            ph = pshp.tile([128, 2, 256], F32,
                                           name=f"ph_{bp}_{step}_{nt}_{cb}",
                                           tag="psh")
                            for jtp in range(JTP):
                                nc.tensor.matmul(
                                    ph[:], st8[jtp][:, :, nsl],
                                    hsrc[jtp][:, :, cb, :, :],
                                    start=(jtp == 0),
                                    stop=(jtp == JTP - 1 and not pe_add),
                                    perf_mode=DR)
                            if pe_add:
                                addend = (
                                    u12t[bp][madd - 1][nt][:, 2, :, :]
                                    if step > 0 else
                                    u0t[bp][nt][:, :, 512:768])
                                nc.tensor.matmul(
                                    ph[:], eye_bf[:], addend,
                                    start=False, stop=True)
                            if step > 0:
                                hd = hdst[nt // 2][:, nt % 2, cb, :, :]
                                if pe_add:
                                    # h_new = psum/256  (fp8 out, ACT)
                                    nc.scalar.mul(hd, ph[:], 1.0 / SS)
                                else:
                                    # h_new = psum/256 + 8*U_madd (DVE)
                                    nc.vector.scalar_tensor_tensor(
                                        hd, ph[:], 1.0 / SS,
                                        u12t[bp][madd - 1][nt][:, cb, :, :],
                                        ALU.mult, ALU.add)
                            else:
                                so = stgp.tile([128, 2, 256], F32,
                                               name=f"so_{bp}_{nt}_{cb}",
                                               tag="outst")
                                if pe_add:
                                    nc.scalar.mul(so[:], ph[:],
                                                  1.0 / (SS * SX * SV))
                                else:
                                    # out = psum/2048 + (U0 + bias)
                                    nc.vector.scalar_tensor_tensor(
                                        so[:], ph[:], 1.0 / (SS * SX * SV),
                                        u0t[bp][nt][:, :,
                                                    cb * 256:(cb + 1) * 256],
                                        ALU.mult, ALU.add)
                                r0 = (2 * bp * JT + nt) * 128
                                nc.sync.dma_start(
                                    out_d.rearrange(
                                        "(x p) e -> p x e", p=128)[
                                        :, r0 // 128:r0 // 128 + 5:4,
                                        cb * 256:(cb + 1) * 256],
                                    so[:])
                    hsrc = hdst

            proj(0)
            proj(1)
            proj(2)
            horner(0)
            proj(3)
            horner(1)
    nc.compile()
    return nc


_CACHE = {}


def _get_program():
    if "nc" not in _CACHE:
        _CACHE["nc"] = _build_program()
    return _CACHE["nc"]


def _e4(x):
    return np.clip(x, -240.0, 240.0).astype(ml_dtypes.float8_e4m3)


def _planepair(a):
    """[768, X] -> [384, 2X]: row = dtp*128+p, col = plane*X + x."""
    x = a.shape[1]
    return np.ascontiguousarray(
        a.reshape(DTP, 2, 128, x).transpose(0, 2, 1, 3).reshape(DTP * 128,
                                                                2 * x))


def make_in_maps(inputs, adj, weights, biases):
    inputs = np.ascontiguousarray(inputs, dtype=np.float32)
    adj = np.ascontiguousarray(adj, dtype=np.float32)
    weights = np.ascontiguousarray(weights, dtype=np.float32)
    biases = np.ascontiguousarray(biases, dtype=np.float32)
    assert inputs.shape == (B, N, D)
    assert adj.shape == (N, N)
    assert weights.shape == (D * 4, D)
    assert biases.shape == (D,)
    eye = np.eye(128, dtype=np.float32)

    Wm = weights.reshape(D, 4, D).transpose(1, 0, 2)  # [m, d, e]
    V0 = Wm[0] - Wm[2]
    V1 = Wm[1] - 3.0 * Wm[3]
    V2 = 2.0 * Wm[2]
    V3 = 4.0 * Wm[3]
    v0 = np.ascontiguousarray(V0).astype(ml_dtypes.bfloat16)
    vcat = np.concatenate(
        [V1[:, 0:512], V2[:, 0:512], V3[:, 0:512],
         V1[:, 512:768], V2[:, 512:768], V3[:, 512:768]], axis=1) * SV
    v8 = _planepair(_e4(vcat))

    in_maps = []
    for c in range(N_CORES):
        x0T = np.ascontiguousarray(
            inputs[c * BL:(c + 1) * BL].reshape(BN, D).T)
        in_maps.append({
            "inp0": x0T.astype(ml_dtypes.bfloat16),
            "inp8": _planepair(_e4(x0T * SX)),
            "v0": v0,
            "v8": v8,
            "adj": adj,
            "bias": biases,
            "eye": eye,
        })
    return in_maps


def kernel(inputs, adj, weights, biases):
    nc = _get_program()
    in_maps = make_in_maps(inputs, adj, weights, biases)
    res = run_bass_kernel_spmd(nc, in_maps, list(range(N_CORES)))
    out = np.concatenate(
        [res.results[c]["out"].reshape(BL, N, D) for c in range(N_CORES)],
        axis=0)
    return out


# revision 12
# speedup vs baseline: 1.1198x; 1.0053x over previous
"""DGCN diffusion-graph-conv kernel for 8 Trainium2 NeuronCores.

Math (per the reference):
    support S = D^-1/2 (adj+I)^T D^-1/2  with D = diag(rowsum(adj+I))
    x_m = T_m(S) x0  (Chebyshev recurrence, K=3 -> m=0..3)
    out = sum_m x_m @ W_m + bias

Folded Chebyshev coefficients:
    V0 = W0 - W2, V1 = W1 - 3*W3, V2 = 2*W2, V3 = 4*W3
    U_m = x0 @ V_m;  out = U0 + S*(U1 + S*(U2 + S*U3))   (Horner)

Mixed precision (data-parallel over batch, 4 batches/core):
    - U0 (carries the full-magnitude signal): bf16 matmuls, full PE rate.
    - U1..U3 projections and the three S-contractions: fp8e4 matmuls in
      DoubleRow mode (two contraction planes per PE cell -> ~1.7x rate).
      Their errors are contracted by S (sigma_2(S) ~ 0.05) before they
      reach the output; measured rel_max ~8e-3 vs the 2e-2 gate.
    - fp8 operands are pre-scaled by powers of two (x0 x4, V123 x2,
      S x256, h-domain x8) chosen so every U-projection PSUM lands
      directly in the fp8 "h domain" (scale 8) and evictions are plain
      copies; Horner evictions fold the 1/256 S-scale into one
      scalar_tensor_tensor.  All values stay below the TRN fp8e4 max
      of +-240.
"""

import numpy as np
import ml_dtypes

import concourse.bacc as bacc
import concourse.tile as tile
import concourse.mybir as mybir
from concourse.bass_utils import run_bass_kernel_spmd

F32 = mybir.dt.float32
BF16 = mybir.dt.bfloat16
F8 = mybir.dt.float8e4
AX = mybir.AxisListType
ALU = mybir.AluOpType
DR = mybir.MatmulPerfMode.DoubleRow

N_CORES = 8
B, N, D = 32, 512, 768
BL = B // N_CORES          # local batches per core = 4
BN = BL * N                # local rows = 2048
DT = D // 128              # 6 feature tiles (bf16 path)
DTP = DT // 2              # 3 feature plane-pairs (fp8 path)
JT = N // 128              # 4 node tiles
JTP = JT // 2              # 2 node plane-pairs
CB = 3                     # 256-wide column blocks per 768 output cols
VCOLS = 3 * D              # 2304 concatenated V123 columns

SX = 4.0                   # x0 fp8 scale
SV = 2.0                   # V123 fp8 scale (SX*SV = 8 = h-domain scale)
SS = 256.0                 # S fp8 scale

# U123 column chunks over the host-permuted 2304-wide Vcat:
#   [V1 e0:512 | V2 e0:512 | V3 e0:512 | V1 e512:768 , V2 e512:768 |
#    V3 e512:768]
# (m, cb0, ncb) per chunk; chunk width = 256*ncb
U123_CHUNKS = [
    (0, [(1, 0, 2)]),          # cols    0:512  -> u1 cb0-1
    (512, [(2, 0, 2)]),        # cols  512:1024 -> u2 cb0-1
    (1024, [(3, 0, 2)]),       # cols 1024:1536 -> u3 cb0-1
    (1536, [(1, 2, 1), (2, 2, 1)]),   # u1 cb2 + u2 cb2
    (2048, [(3, 2, 1)]),       # u3 cb2
]


def _build_program():
    nc = bacc.Bacc("TRN2", target_bir_lowering=False, debug=False,
                   num_devices=N_CORES)
    inp0_d = nc.dram_tensor("inp0", [D, BN], BF16, kind="ExternalInput").ap()
    inp8_d = nc.dram_tensor("inp8", [DTP * 128, 2 * BN], F8,
                            kind="ExternalInput").ap()
    v0_d = nc.dram_tensor("v0", [D, D], BF16, kind="ExternalInput").ap()
    v8_d = nc.dram_tensor("v8", [DTP * 128, 2 * VCOLS], F8,
                          kind="ExternalInput").ap()
    adj_d = nc.dram_tensor("adj", [N, N], F32, kind="ExternalInput").ap()
    bias_d = nc.dram_tensor("bias", [D], F32, kind="ExternalInput").ap()
    eye_d = nc.dram_tensor("eye", [128, 128], F32, kind="ExternalInput").ap()
    out_d = nc.dram_tensor("out", [BN, D], F32, kind="ExternalOutput").ap()
    dscr = nc.dram_tensor("dscr", [N], F32)

    with tile.TileContext(nc) as tc:
        with (
            tc.tile_pool(name="const", bufs=1) as constp,
            tc.tile_pool(name="sup", bufs=1) as supp,
            tc.tile_pool(name="x0", bufs=1) as x0p,
            tc.tile_pool(name="x8", bufs=1) as x8p,
            tc.tile_pool(name="v0", bufs=1) as v0p,
            tc.tile_pool(name="v8", bufs=1) as v8p,
            tc.tile_pool(name="u0", bufs=1) as u0p,
            tc.tile_pool(name="u12", bufs=1) as u12p,
            tc.tile_pool(name="u3", bufs=1) as u3p,
            tc.tile_pool(name="h", bufs=4) as hp,
            tc.tile_pool(name="stg", bufs=2) as stgp,
            tc.tile_pool(name="ps0", bufs=2, space="PSUM") as ps0p,
            tc.tile_pool(name="ps8", bufs=3, space="PSUM") as ps8p,
            tc.tile_pool(name="psh", bufs=3, space="PSUM") as pshp,
        ):
            # ---- input DMAs, first-needed first; fp8 operands (small)
            #      on the sync ring, bf16/f32 on the gpsimd ring ----
            v0t = [v0p.tile([128, D], BF16, name=f"v0_{dt}")
                   for dt in range(DT)]
            x0t = [x0p.tile([128, BN], BF16, name=f"x0_{dt}")
                   for dt in range(DT)]
            v8t = [v8p.tile([128, 2, VCOLS], F8, name=f"v8_{dtp}")
                   for dtp in range(DTP)]
            x8t = [x8p.tile([128, 2, BN], F8, name=f"x8_{dtp}")
                   for dtp in range(DTP)]
            eye128 = constp.tile([128, 128], F32)
            nc.gpsimd.dma_start(eye128[:], eye_d[:])
            adjts = []
            for t in range(JT):
                adjt = supp.tile([128, N], F32, name=f"adjt{t}")
                nc.gpsimd.dma_start(adjt[:], adj_d[t * 128:(t + 1) * 128, :])
                adjts.append(adjt)
            # sync ring carries the bulk inputs, first-needed first, in
            # few large descriptors (each descriptor costs ~650ns of
            # ring time regardless of size):
            # v8 whole, x8 batch0, v0, x0 batch0, x8 rest, x0 rest
            inp8_v = [inp8_d[dtp * 128:(dtp + 1) * 128, :].rearrange(
                "p (i r) -> p i r", i=2) for dtp in range(DTP)]
            for dtp in range(DTP):
                nc.sync.dma_start(v8t[dtp][:],
                                  v8_d[dtp * 128:(dtp + 1) * 128, :])
            for dtp in range(DTP):
                nc.sync.dma_start(x8t[dtp][:, :, 0:512],
                                  inp8_v[dtp][:, :, 0:512])
            for dt in range(DT):
                nc.sync.dma_start(v0t[dt][:],
                                  v0_d[dt * 128:(dt + 1) * 128, :])
            for dt in range(DT):
                nc.sync.dma_start(x0t[dt][:, 0:512],
                                  inp0_d[dt * 128:(dt + 1) * 128, 0:512])
            for dtp in range(DTP):
                nc.sync.dma_start(x8t[dtp][:, :, 512:BN],
                                  inp8_v[dtp][:, :, 512:BN])
            for dt in range(DT):
                nc.sync.dma_start(x0t[dt][:, 512:BN],
                                  inp0_d[dt * 128:(dt + 1) * 128, 512:BN])
            bias_bc = constp.tile([128, D], F32)
            nc.gpsimd.dma_start(
                bias_bc[:], bias_d.unsqueeze(0).broadcast_to([128, D]))

            # ---- support matrix S^T (f32 build as baseline, then x256
            #      quantize into fp8 plane-pair tiles) ----
            dcols, dsqs = [], []
            for t in range(JT):
                rs = supp.tile([128, 1], F32, name=f"rs{t}", tag="rs", bufs=2)
                nc.vector.tensor_reduce(rs[:], adjts[t][:], axis=AX.X,
                                        op=ALU.add)
                nc.vector.tensor_scalar_add(rs[:], rs[:], 1.0)
                sq = supp.tile([128, 1], F32, name=f"sq{t}", tag="sq", bufs=2)
                nc.scalar.sqrt(sq[:], rs[:])
                dcol = supp.tile([128, 1], F32, name=f"dcol{t}")
                nc.vector.reciprocal(dcol[:], sq[:])
                dsq = supp.tile([128, 1], F32, name=f"dsq{t}")
                nc.vector.tensor_mul(dsq[:], dcol[:], dcol[:])
                nc.gpsimd.dma_start(dscr.ap()[t * 128:(t + 1) * 128], dcol[:])
                dcols.append(dcol)
                dsqs.append(dsq)
            dbc = constp.tile([128, N], F32)
            nc.gpsimd.dma_start(
                dbc[:], dscr.ap().unsqueeze(0).broadcast_to([128, N]))
            st8 = [supp.tile([128, 2, N], F8, name=f"st8_{jtp}")
                   for jtp in range(JTP)]
            for t in range(JT):
                s = supp.tile([128, N], F32, name=f"st{t}", tag="stf",
                              bufs=2)
                nc.vector.scalar_tensor_tensor(
                    s[:], adjts[t][:], dcols[t][:], dbc[:],
                    ALU.mult, ALU.mult)
                diagfix = supp.tile([128, 128], F32, name=f"dfix{t}",
                                    tag="dfix", bufs=2)
                nc.vector.tensor_scalar_mul(diagfix[:], eye128[:], dsqs[t][:])
                nc.vector.tensor_add(
                    s[:, t * 128:(t + 1) * 128],
                    s[:, t * 128:(t + 1) * 128], diagfix[:])
                nc.vector.tensor_scalar_mul(st8[t // 2][:, t % 2, :], s[:],
                                            SS)
            # bf16 identity + 2048*bias[512:768] for the tail's
            # PE-side addend path (batch pair 1, cb2 groups)
            eye_bf = constp.tile([128, 128], BF16)
            nc.vector.tensor_copy(eye_bf[:], eye128[:])
            b2k = constp.tile([128, 256], F32)
            nc.vector.tensor_scalar_mul(b2k[:], bias_bc[:, 512:768], 2048.0)

            # ---- per-batch-pair U tiles ----
            # u0: [row128, batch-parity, 768] bf16 (holds U0 + bias)
            # u12: [row128, cb, batch-parity, 256] bf16 (holds 8*U_m)
            # u3/h: [row128, j-plane, cb, batch-parity, 256] fp8 (8*U3 / 8*h)
            u0t = [[u0p.tile([128, 2, D], BF16, name=f"u0_{bp}_{nt}")
                    for nt in range(JT)] for bp in range(2)]
            u12t = [[[u12p.tile([128, CB, 2, 256], BF16,
                                name=f"u{m}_{bp}_{nt}")
                      for nt in range(JT)] for m in (1, 2)]
                    for bp in range(2)]
            u3t = [[u3p.tile([128, 2, CB, 2, 256], F8,
                             name=f"u3_{bp}_{jtp}")
                    for jtp in range(JTP)] for bp in range(2)]

            def proj(b):
                """U123 (fp8 DoubleRow) + U0 (bf16) for batch b.

                For batch pair 1, the cb2 slices of u1/u2/u0 are stored in
                the 2048 (PSUM) domain so the Horner tail can add them via
                identity-matmul accumulation on the PE."""
                bp, h = b // 2, b % 2
                for nt in range(JT):
                    bt = b * JT + nt
                    rsl = slice(bt * 128, (bt + 1) * 128)
                    # U123: fp8 DoubleRow over host-permuted Vcat chunks
                    for c0, dests in U123_CHUNKS:
                        cw = 256 * sum(ncb for _, _, ncb in dests)
                        ps = ps8p.tile([128, 512], F32,
                                       name=f"p8_{bt}_{c0}", tag="ps8")
                        for dtp in range(DTP):
                            nc.tensor.matmul(
                                ps[:, 0:cw], x8t[dtp][:, :, rsl],
                                v8t[dtp][:, :, c0:c0 + cw],
                                start=(dtp == 0), stop=(dtp == DTP - 1),
                                perf_mode=DR)
                        off = 0
                        for m, cb0, ncb in dests:
                            w = 256 * ncb
                            src = ps[:, off:off + w]
                            if m == 3:
                                nc.scalar.copy(
                                    u3t[bp][nt // 2][
                                        :, nt % 2, cb0:cb0 + ncb, h, :],
                                    src)
                            elif bp == 1 and cb0 == 2:
                                nc.scalar.mul(
                                    u12t[bp][m - 1][nt][
                                        :, cb0:cb0 + ncb, h, :],
                                    src, SS)
                            else:
                                nc.scalar.copy(
                                    u12t[bp][m - 1][nt][
                                        :, cb0:cb0 + ncb, h, :],
                                    src)
                            off += w
                    # U0: full-rate bf16, cols 768 in (512, 256) chunks
                    for c0, cw in ((0, 512), (512, 256)):
                        ps = ps0p.tile([128, 512], F32,
                                       name=f"p0_{bt}_{c0}", tag="ps0")
                        for dt in range(DT):
                            nc.tensor.matmul(
                                ps[:, 0:cw], x0t[dt][:, rsl],
                                v0t[dt][:, c0:c0 + cw],
                                start=(dt == 0), stop=(dt == DT - 1))
                        if bp == 1 and c0 == 512:
                            nc.vector.scalar_tensor_tensor(
                                u0t[bp][nt][:, h, c0:c0 + cw], ps[:, 0:cw],
                                2048.0, b2k[:], ALU.mult, ALU.add)
                        else:
                            nc.vector.tensor_add(
                                u0t[bp][nt][:, h, c0:c0 + cw], ps[:, 0:cw],
                                bias_bc[:, c0:c0 + cw])

            def horner(bp):
                """out = U0 + S*(U1 + S*(U2 + S*U3)) for batch pair bp."""
                hsrc = u3t[bp]
                for step, madd in ((2, 2), (1, 1), (0, 0)):
                    hdst = None
                    if step > 0:
                        hdst = [hp.tile([128, 2, CB, 2, 256], F8,
                                        name=f"h_{bp}_{step}_{jtp}",
                                        tag="h")
                                for jtp in range(JTP)]
                    for nt in range(JT):
                        nsl = slice(nt * 128, (nt + 1) * 128)
                        so = None
                        if step == 0:
                            so = stgp.tile([128, 2, D], F32,
                                           name=f"so_{bp}_{nt}", tag="outst")
                        for cb in range(CB):
                            # batch pair 1 cb2: U-addend joins via
                            # identity-matmul so the evict is a pure scaled
                            # copy that runs on the Scalar engine (keeps
                            # the tail off the DVE critical path)
                            pe_add = bp == 1 and cb == 2
                            ph = pshp.tile([128, 2, 256], F32,
                                           name=f"ph_{bp}_{step}_{nt}_{cb}",
                                           tag="psh")
                            for jtp in range(JTP):
                                nc.tensor.matmul(
                                    ph[:], st8[jtp][:, :, nsl],
                                    hsrc[jtp][:, :, cb, :, :],
                                    start=(jtp == 0),
                                    stop=(jtp == JTP - 1 and not pe_add),
                                    perf_mode=DR)
                            if pe_add:
                                addend = (
                                    u12t[bp][madd - 1][nt][:, 2, :, :]
                                    if step > 0 else
                                    u0t[bp][nt][:, :, 512:768])
                                nc.tensor.matmul(
                                    ph[:], eye_bf[:], addend,
                                    start=False, stop=True)
                            if step > 0:
                                hd = hdst[nt // 2][:, nt % 2, cb, :, :]
                                if pe_add:
                                    # h_new = psum/256  (fp8 out, ACT)
                                    nc.scalar.mul(hd, ph[:], 1.0 / SS)
                                else:
                                    # h_new = psum/256 + 8*U_madd (DVE)
                                    nc.vector.scalar_tensor_tensor(
                                        hd, ph[:], 1.0 / SS,
                                        u12t[bp][madd - 1][nt][:, cb, :, :],
                                        ALU.mult, ALU.add)
                            else:
                                cs = slice(cb * 256, (cb + 1) * 256)
                                if pe_add:
                                    nc.scalar.mul(so[:, :, cs], ph[:],
                                                  1.0 / (SS * SX * SV))
                                else:
                                    # out = psum/2048 + (U0 + bias)
                                    nc.vector.scalar_tensor_tensor(
                                        so[:, :, cs], ph[:],
                                        1.0 / (SS * SX * SV),
                                        u0t[bp][nt][:, :, cs],
                                        ALU.mult, ALU.add)
                        if step == 0:
                            r0 = (2 * bp * JT + nt) * 128
                            nc.sync.dma_start(
                                out_d.rearrange(
                                    "(x p) e -> p x e", p=128)[
                                    :, r0 // 128:r0 // 128 + 5:4, :],
                                so[:])
                    hsrc = hdst

            proj(0)
            proj(1)
            proj(2)
            horner(0)
            proj(3)
            horner(1)
    nc.compile()
    return nc


_CACHE = {}


def _get_program():
    if "nc" not in _CACHE:
        _CACHE["nc"] = _build_program()
    return _CACHE["nc"]


def _e4(x):
    return np.clip(x, -240.0, 240.0).astype(ml_dtypes.float8_e4m3)


def _planepair(a):
    """[768, X] -> [384, 2X]: row = dtp*128+p, col = plane*X + x."""
    x = a.shape[1]
    return np.ascontiguousarray(
        a.reshape(DTP, 2, 128, x).transpose(0, 2, 1, 3).reshape(DTP * 128,
                                                                2 * x))


def make_in_maps(inputs, adj, weights, biases):
    inputs = np.ascontiguousarray(inputs, dtype=np.float32)
    adj = np.ascontiguousarray(adj, dtype=np.float32)
    weights = np.ascontiguousarray(weights, dtype=np.float32)
    biases = np.ascontiguousarray(biases, dtype=np.float32)
    assert inputs.shape == (B, N, D)
    assert adj.shape == (N, N)
    assert weights.shape == (D * 4, D)
    assert biases.shape == (D,)
    eye = np.eye(128, dtype=np.float32)

    Wm = weights.reshape(D, 4, D).transpose(1, 0, 2)  # [m, d, e]
    V0 = Wm[0] - Wm[2]
    V1 = Wm[1] - 3.0 * Wm[3]
    V2 = 2.0 * Wm[2]
    V3 = 4.0 * Wm[3]
    v0 = np.ascontiguousarray(V0).astype(ml_dtypes.bfloat16)
    vcat = np.concatenate(
        [V1[:, 0:512], V2[:, 0:512], V3[:, 0:512],
         V1[:, 512:768], V2[:, 512:768], V3[:, 512:768]], axis=1) * SV
    v8 = _planepair(_e4(vcat))

    in_maps = []
    for c in range(N_CORES):
        x0T = np.ascontiguousarray(
            inputs[c * BL:(c + 1) * BL].reshape(BN, D).T)
        in_maps.append({
            "inp0": x0T.astype(ml_dtypes.bfloat16),
            "inp8": _planepair(_e4(x0T * SX)),
            "v0": v0,
            "v8": v8,
            "adj": adj,
            "bias": biases,
            "eye": eye,
        })
    return in_maps


def kernel(inputs, adj, weights, biases):
    nc = _get_program()
    in_maps = make_in_maps(inputs, adj, weights, biases)
    res = run_bass_kernel_spmd(nc, in_maps, list(range(N_CORES)))
    out = np.concatenate(
        [res.results[c]["out"].reshape(BL, N, D) for c in range(N_CORES)],
        axis=0)
    return out


# revision 16
# speedup vs baseline: 1.1352x; 1.0137x over previous
"""DGCN diffusion-graph-conv kernel for 8 Trainium2 NeuronCores.

Math (per the reference):
    support S = D^-1/2 (adj+I)^T D^-1/2  with D = diag(rowsum(adj+I))
    x_m = T_m(S) x0  (Chebyshev recurrence, K=3 -> m=0..3)
    out = sum_m x_m @ W_m + bias

Folded Chebyshev coefficients:
    V0 = W0 - W2, V1 = W1 - 3*W3, V2 = 2*W2, V3 = 4*W3
    U_m = x0 @ V_m;  out = U0 + S*(U1 + S*(U2 + S*U3))   (Horner)

Mixed precision (data-parallel over batch, 4 batches/core):
    - U0 (carries the full-magnitude signal): bf16 matmuls, full PE rate.
    - U1..U3 projections and the three S-contractions: fp8e4 matmuls in
      DoubleRow mode (two contraction planes per PE cell -> ~1.7x rate).
      Their errors are contracted by S (sigma_2(S) ~ 0.05) before they
      reach the output; measured rel_max ~8e-3 vs the 2e-2 gate.
    - fp8 operands are pre-scaled by powers of two (x0 x4, V123 x2,
      S x256, h-domain x8) chosen so every U-projection PSUM lands
      directly in the fp8 "h domain" (scale 8) and evictions are plain
      copies; Horner evictions fold the 1/256 S-scale into one
      scalar_tensor_tensor.  All values stay below the TRN fp8e4 max
      of +-240.
"""

import numpy as np
import ml_dtypes

import concourse.bacc as bacc
import concourse.tile as tile
import concourse.mybir as mybir
from concourse.bass_utils import run_bass_kernel_spmd

F32 = mybir.dt.float32
BF16 = mybir.dt.bfloat16
F8 = mybir.dt.float8e4
AX = mybir.AxisListType
ALU = mybir.AluOpType
DR = mybir.MatmulPerfMode.DoubleRow

N_CORES = 8
B, N, D = 32, 512, 768
BL = B // N_CORES          # local batches per core = 4
BN = BL * N                # local rows = 2048
DT = D // 128              # 6 feature tiles (bf16 path)
DTP = DT // 2              # 3 feature plane-pairs (fp8 path)
JT = N // 128              # 4 node tiles
JTP = JT // 2              # 2 node plane-pairs
CB = 3                     # 256-wide column blocks per 768 output cols
VCOLS = 3 * D              # 2304 concatenated V123 columns

SX = 4.0                   # x0 fp8 scale
SV = 2.0                   # V123 fp8 scale (SX*SV = 8 = h-domain scale)
SS = 256.0                 # S fp8 scale

# U123 column chunks over the host-permuted 2304-wide Vcat:
#   [V1 e0:512 | V2 e0:512 | V3 e0:512 | V1 e512:768 , V2 e512:768 |
#    V3 e512:768]
# (m, cb0, ncb) per chunk; chunk width = 256*ncb.  m=3 chunks run first
# so u3 (the next Horner stage's operand) is produced earliest.
U123_CHUNKS = [
    (1024, [(3, 0, 2)]),       # cols 1024:1536 -> u3 cb0-1
    (2048, [(3, 2, 1)]),       # u3 cb2
    (1536, [(1, 2, 1), (2, 2, 1)]),   # u1 cb2 + u2 cb2
    (0, [(1, 0, 2)]),          # cols    0:512  -> u1 cb0-1
    (512, [(2, 0, 2)]),        # cols  512:1024 -> u2 cb0-1
]


def _build_program():
    nc = bacc.Bacc("TRN2", target_bir_lowering=False, debug=False,
                   num_devices=N_CORES)
    inp0_d = nc.dram_tensor("inp0", [D, BN], BF16, kind="ExternalInput").ap()
    inp8_d = nc.dram_tensor("inp8", [DTP * 128, 2 * BN], F8,
                            kind="ExternalInput").ap()
    v0_d = nc.dram_tensor("v0", [D, D], BF16, kind="ExternalInput").ap()
    v8_d = nc.dram_tensor("v8", [DTP * 128, 2 * VCOLS], F8,
                          kind="ExternalInput").ap()
    adj_d = nc.dram_tensor("adj", [N, N], F32, kind="ExternalInput").ap()
    bias_d = nc.dram_tensor("bias", [D], F32, kind="ExternalInput").ap()
    eye_d = nc.dram_tensor("eye", [128, 128], F32, kind="ExternalInput").ap()
    out_d = nc.dram_tensor("out", [BN, D], F32, kind="ExternalOutput").ap()
    dscr = nc.dram_tensor("dscr", [N], F32)

    with tile.TileContext(nc) as tc:
        with (
            tc.tile_pool(name="const", bufs=1) as constp,
            tc.tile_pool(name="sup", bufs=1) as supp,
            tc.tile_pool(name="x0", bufs=1) as x0p,
            tc.tile_pool(name="x8", bufs=1) as x8p,
            tc.tile_pool(name="v0", bufs=1) as v0p,
            tc.tile_pool(name="v8", bufs=1) as v8p,
            tc.tile_pool(name="u0", bufs=1) as u0p,
            tc.tile_pool(name="u12", bufs=1) as u12p,
            tc.tile_pool(name="u3", bufs=1) as u3p,
            tc.tile_pool(name="h", bufs=4) as hp,
            tc.tile_pool(name="stg", bufs=2) as stgp,
            tc.tile_pool(name="ps0", bufs=2, space="PSUM") as ps0p,
            tc.tile_pool(name="ps8", bufs=3, space="PSUM") as ps8p,
            tc.tile_pool(name="psh", bufs=3, space="PSUM") as pshp,
        ):
            # ---- input DMAs, first-needed first; fp8 operands (small)
            #      on the sync ring, bf16/f32 on the gpsimd ring ----
            v0t = [v0p.tile([128, D], BF16, name=f"v0_{dt}")
                   for dt in range(DT)]
            x0t = [x0p.tile([128, BN], BF16, name=f"x0_{dt}")
                   for dt in range(DT)]
            v8t = [v8p.tile([128, 2, VCOLS], F8, name=f"v8_{dtp}")
                   for dtp in range(DTP)]
            x8t = [x8p.tile([128, 2, BN], F8, name=f"x8_{dtp}")
                   for dtp in range(DTP)]
            # Bulk inputs in few large descriptors (each descriptor costs
            # ~650ns of ring time regardless of size), first-needed first.
            # sync ring: v8 cols 1024: (the m3-first chunks), v8 cols
            # :1024, v0, x0 batch0, x8 rest, x0 rest; the small x8-batch0
            # slices load in parallel on the gpsimd ring ahead of adj.
            inp8_v = [inp8_d[dtp * 128:(dtp + 1) * 128, :].rearrange(
                "p (i r) -> p i r", i=2) for dtp in range(DTP)]
            v8_v = [v8_d[dtp * 128:(dtp + 1) * 128, :].rearrange(
                "p (i c) -> p i c", i=2) for dtp in range(DTP)]
            for dtp in range(DTP):
                nc.gpsimd.dma_start(x8t[dtp][:, :, 0:512],
                                    inp8_v[dtp][:, :, 0:512])
            eye128 = constp.tile([128, 128], F32)
            nc.gpsimd.dma_start(eye128[:], eye_d[:])
            adjts = []
            for t in range(JT):
                adjt = supp.tile([128, N], F32, name=f"adjt{t}")
                nc.gpsimd.dma_start(adjt[:], adj_d[t * 128:(t + 1) * 128, :])
                adjts.append(adjt)
            for dtp in range(DTP):
                nc.sync.dma_start(v8t[dtp][:, :, 1024:VCOLS],
                                  v8_v[dtp][:, :, 1024:VCOLS])
            for dtp in range(DTP):
                nc.sync.dma_start(v8t[dtp][:, :, 0:1024],
                                  v8_v[dtp][:, :, 0:1024])
            for dt in range(DT):
                nc.sync.dma_start(v0t[dt][:],
                                  v0_d[dt * 128:(dt + 1) * 128, :])
            for dt in range(DT):
                nc.sync.dma_start(x0t[dt][:, 0:512],
                                  inp0_d[dt * 128:(dt + 1) * 128, 0:512])
            for dtp in range(DTP):
                nc.sync.dma_start(x8t[dtp][:, :, 512:BN],
                                  inp8_v[dtp][:, :, 512:BN])
            for dt in range(DT):
                nc.sync.dma_start(x0t[dt][:, 512:BN],
                                  inp0_d[dt * 128:(dt + 1) * 128, 512:BN])
            bias_bc = constp.tile([128, D], F32)
            nc.gpsimd.dma_start(
                bias_bc[:], bias_d.unsqueeze(0).broadcast_to([128, D]))

            # ---- support matrix S^T (f32 build as baseline, then x256
            #      quantize into fp8 plane-pair tiles) ----
            dcols, dsqs = [], []
            for t in range(JT):
                rs = supp.tile([128, 1], F32, name=f"rs{t}", tag="rs", bufs=2)
                nc.vector.tensor_reduce(rs[:], adjts[t][:], axis=AX.X,
                                        op=ALU.add)
                nc.vector.tensor_scalar_add(rs[:], rs[:], 1.0)
                sq = supp.tile([128, 1], F32, name=f"sq{t}", tag="sq", bufs=2)
                nc.scalar.sqrt(sq[:], rs[:])
                dcol = supp.tile([128, 1], F32, name=f"dcol{t}")
                nc.vector.reciprocal(dcol[:], sq[:])
                dsq = supp.tile([128, 1], F32, name=f"dsq{t}")
                nc.vector.tensor_mul(dsq[:], dcol[:], dcol[:])
                nc.gpsimd.dma_start(dscr.ap()[t * 128:(t + 1) * 128], dcol[:])
                dcols.append(dcol)
                dsqs.append(dsq)
            dbc = constp.tile([128, N], F32)
            nc.gpsimd.dma_start(
                dbc[:], dscr.ap().unsqueeze(0).broadcast_to([128, N]))
            st8 = [supp.tile([128, 2, N], F8, name=f"st8_{jtp}")
                   for jtp in range(JTP)]
            for t in range(JT):
                s = supp.tile([128, N], F32, name=f"st{t}", tag="stf",
                              bufs=2)
                nc.vector.scalar_tensor_tensor(
                    s[:], adjts[t][:], dcols[t][:], dbc[:],
                    ALU.mult, ALU.mult)
                diagfix = supp.tile([128, 128], F32, name=f"dfix{t}",
                                    tag="dfix", bufs=2)
                nc.vector.tensor_scalar_mul(diagfix[:], eye128[:], dsqs[t][:])
                nc.vector.tensor_add(
                    s[:, t * 128:(t + 1) * 128],
                    s[:, t * 128:(t + 1) * 128], diagfix[:])
                nc.vector.tensor_scalar_mul(st8[t // 2][:, t % 2, :], s[:],
                                            SS)
            # bf16 identity + 2048*bias[512:768] for the tail's
            # PE-side addend path (batch pair 1, cb2 groups)
            eye_bf = constp.tile([128, 128], BF16)
            nc.vector.tensor_copy(eye_bf[:], eye128[:])
            b2k = constp.tile([128, 256], F32)
            nc.vector.tensor_scalar_mul(b2k[:], bias_bc[:, 512:768], 2048.0)

            # ---- per-batch-pair U tiles ----
            # u0: [row128, batch-parity, 768] bf16 (holds U0 + bias)
            # u12: [row128, cb, batch-parity, 256] bf16 (holds 8*U_m)
            # u3/h: [row128, j-plane, cb, batch-parity, 256] fp8 (8*U3 / 8*h)
            u0t = [[u0p.tile([128, 2, D], BF16, name=f"u0_{bp}_{nt}")
                    for nt in range(JT)] for bp in range(2)]
            u12t = [[[u12p.tile([128, CB, 2, 256], BF16,
                                name=f"u{m}_{bp}_{nt}")
                      for nt in range(JT)] for m in (1, 2)]
                    for bp in range(2)]
            u3t = [[u3p.tile([128, 2, CB, 2, 256], F8,
                             name=f"u3_{bp}_{jtp}")
                    for jtp in range(JTP)] for bp in range(2)]

            def proj(b):
                """U123 (fp8 DoubleRow) + U0 (bf16) for batch b.

                For batch pair 1, the cb2 slices of u1/u2/u0 are stored in
                the 2048 (PSUM) domain so the Horner tail can add them via
                identity-matmul accumulation on the PE."""
                bp, h = b // 2, b % 2
                for nt in range(JT):
                    bt = b * JT + nt
                    rsl = slice(bt * 128, (bt + 1) * 128)
                    # U123: fp8 DoubleRow over host-permuted Vcat chunks
                    for c0, dests in U123_CHUNKS:
                        cw = 256 * sum(ncb for _, _, ncb in dests)
                        ps = ps8p.tile([128, 512], F32,
                                       name=f"p8_{bt}_{c0}", tag="ps8")
                        for dtp in range(DTP):
                            nc.tensor.matmul(
                                ps[:, 0:cw], x8t[dtp][:, :, rsl],
                                v8t[dtp][:, :, c0:c0 + cw],
                                start=(dtp == 0), stop=(dtp == DTP - 1),
                                perf_mode=DR)
                        off = 0
                        for m, cb0, ncb in dests:
                            w = 256 * ncb
                            src = ps[:, off:off + w]
                            if m == 3:
                                nc.scalar.copy(
                                    u3t[bp][nt // 2][
                                        :, nt % 2, cb0:cb0 + ncb, h, :],
                                    src)
                            elif bp == 1 and cb0 == 2:
                                nc.scalar.mul(
                                    u12t[bp][m - 1][nt][
                                        :, cb0:cb0 + ncb, h, :],
                                    src, SS)
                            else:
                                nc.scalar.copy(
                                    u12t[bp][m - 1][nt][
                                        :, cb0:cb0 + ncb, h, :],
                                    src)
                            off += w
                    # U0: full-rate bf16, cols 768 in (512, 256) chunks
                    for c0, cw in ((0, 512), (512, 256)):
                        ps = ps0p.tile([128, 512], F32,
                                       name=f"p0_{bt}_{c0}", tag="ps0")
                        for dt in range(DT):
                            nc.tensor.matmul(
                                ps[:, 0:cw], x0t[dt][:, rsl],
                                v0t[dt][:, c0:c0 + cw],
                                start=(dt == 0), stop=(dt == DT - 1))
                        if bp == 1 and c0 == 512:
                            nc.vector.scalar_tensor_tensor(
                                u0t[bp][nt][:, h, c0:c0 + cw], ps[:, 0:cw],
                                2048.0, b2k[:], ALU.mult, ALU.add)
                        else:
                            nc.vector.tensor_add(
                                u0t[bp][nt][:, h, c0:c0 + cw], ps[:, 0:cw],
                                bias_bc[:, c0:c0 + cw])

            def horner(bp):
                """out = U0 + S*(U1 + S*(U2 + S*U3)) for batch pair bp."""
                hsrc = u3t[bp]
                for step, madd in ((2, 2), (1, 1), (0, 0)):
                    hdst = None
                    if step > 0:
                        hdst = [hp.tile([128, 2, CB, 2, 256], F8,
                                        name=f"h_{bp}_{step}_{jtp}",
                                        tag="h")
                                for jtp in range(JTP)]
                    for nt in range(JT):
                        nsl = slice(nt * 128, (nt + 1) * 128)
                        so = None
                        if step == 0:
                            so = stgp.tile([128, 2, D], F32,
                                           name=f"so_{bp}_{nt}", tag="outst")
                        for cb in range(CB):
                            # batch pair 1 cb2: U-addend joins via
                            # identity-matmul so the evict is a pure scaled
                            # copy that runs on the Scalar engine (keeps
                            # the tail off the DVE critical path)
                            pe_add = bp == 1 and cb == 2
                            ph = pshp.tile([128, 2, 256], F32,
                                           name=f"ph_{bp}_{step}_{nt}_{cb}",
                                           tag="psh")
                            for jtp in range(JTP):
                                nc.tensor.matmul(
                                    ph[:], st8[jtp][:, :, nsl],
                                    hsrc[jtp][:, :, cb, :, :],
                                    start=(jtp == 0),
                                    stop=(jtp == JTP - 1 and not pe_add),
                                    perf_mode=DR)
                            if pe_add:
                                addend = (
                                    u12t[bp][madd - 1][nt][:, 2, :, :]
                                    if step > 0 else
                                    u0t[bp][nt][:, :, 512:768])
                                nc.tensor.matmul(
                                    ph[:], eye_bf[:], addend,
                                    start=False, stop=True)
                            if step > 0:
                                hd = hdst[nt // 2][:, nt % 2, cb, :, :]
                                if pe_add:
                                    # h_new = psum/256  (fp8 out, ACT)
                                    nc.scalar.mul(hd, ph[:], 1.0 / SS)
                                else:
                                    # h_new = psum/256 + 8*U_madd (DVE)
                                    nc.vector.scalar_tensor_tensor(
                                        hd, ph[:], 1.0 / SS,
                                        u12t[bp][madd - 1][nt][:, cb, :, :],
                                        ALU.mult, ALU.add)
                            else:
                                cs = slice(cb * 256, (cb + 1) * 256)
                                if pe_add:
                                    nc.scalar.mul(so[:, :, cs], ph[:],
                                                  1.0 / (SS * SX * SV))
                                else:
                                    # out = psum/2048 + (U0 + bias)
                                    nc.vector.scalar_tensor_tensor(
                                        so[:, :, cs], ph[:],
                                        1.0 / (SS * SX * SV),
                                        u0t[bp][nt][:, :, cs],
                                        ALU.mult, ALU.add)
                        if step == 0:
                            r0 = (2 * bp * JT + nt) * 128
                            eng = nc.sync if nt % 2 == 0 else nc.gpsimd
                            eng.dma_start(
                                out_d.rearrange(
                                    "(x p) e -> p x e", p=128)[
                                    :, r0 // 128:r0 // 128 + 5:4, :],
                                so[:])
                    hsrc = hdst

            proj(0)
            proj(1)
            proj(2)
            horner(0)
            proj(3)
            horner(1)
    nc.compile()
    return nc


_CACHE = {}


def _get_program():
    if "nc" not in _CACHE:
        _CACHE["nc"] = _build_program()
    return _CACHE["nc"]


def _e4(x):
    return np.clip(x, -240.0, 240.0).astype(ml_dtypes.float8_e4m3)


def _planepair(a):
    """[768, X] -> [384, 2X]: row = dtp*128+p, col = plane*X + x."""
    x = a.shape[1]
    return np.ascontiguousarray(
        a.reshape(DTP, 2, 128, x).transpose(0, 2, 1, 3).reshape(DTP * 128,
                                                                2 * x))


def make_in_maps(inputs, adj, weights, biases):
    inputs = np.ascontiguousarray(inputs, dtype=np.float32)
    adj = np.ascontiguousarray(adj, dtype=np.float32)
    weights = np.ascontiguousarray(weights, dtype=np.float32)
    biases = np.ascontiguousarray(biases, dtype=np.float32)
    assert inputs.shape == (B, N, D)
    assert adj.shape == (N, N)
    assert weights.shape == (D * 4, D)
    assert biases.shape == (D,)
    eye = np.eye(128, dtype=np.float32)

    Wm = weights.reshape(D, 4, D).transpose(1, 0, 2)  # [m, d, e]
    V0 = Wm[0] - Wm[2]
    V1 = Wm[1] - 3.0 * Wm[3]
    V2 = 2.0 * Wm[2]
    V3 = 4.0 * Wm[3]
    v0 = np.ascontiguousarray(V0).astype(ml_dtypes.bfloat16)
    vcat = np.concatenate(
        [V1[:, 0:512], V2[:, 0:512], V3[:, 0:512],
         V1[:, 512:768], V2[:, 512:768], V3[:, 512:768]], axis=1) * SV
    v8 = _planepair(_e4(vcat))

    in_maps = []
    for c in range(N_CORES):
        x0T = np.ascontiguousarray(
            inputs[c * BL:(c + 1) * BL].reshape(BN, D).T)
        in_maps.append({
            "inp0": x0T.astype(ml_dtypes.bfloat16),
            "inp8": _planepair(_e4(x0T * SX)),
            "v0": v0,
            "v8": v8,
            "adj": adj,
            "bias": biases,
            "eye": eye,
        })
    return in_maps


def kernel(inputs, adj, weights, biases):
    nc = _get_program()
    in_maps = make_in_maps(inputs, adj, weights, biases)
    res = run_bass_kernel_spmd(nc, in_maps, list(range(N_CORES)))
    out = np.concatenate(
        [res.results[c]["out"].reshape(BL, N, D) for c in range(N_CORES)],
        axis=0)
    return out
